# revision 8
# baseline (speedup 1.0000x reference)
"""Masked multi-head attention (CLS-token sparse attention) on 8 Trainium2
NeuronCores, data-parallel over batch (1 batch element per core).

Sparsity: the key mask is query-independent, so masked keys contribute
nothing.  The host gathers the ~513-548 unmasked keys per batch, pads to
NK=576 (4 full 128-key chunks + one 64-key tail), and the device only
computes K/V projections, scores, exp and attention*V over those slots
(padding slots get a -1e9 bias so exp()==0 and the fused denominator
ignores them).

Per-core math (transposed layouts keep every matmul operand natural):
  x^T [c, n] for queries; xkv^T [c, j'] gathered keys.
  q^T = (wq*scale)^T-matmul;  k^T [o, j'];  v [j', o] natural.
  S^T[j', i] = k_h^T.T @ q_h^T   (K=64 on partitions; head pairs run
                                  CONCURRENT via PE row groups 0:64/64:128)
  E = exp(S^T + bias[j'])        (ACT, per-partition bias)
  [O'^T ; denom] = [v_h | 1].T @ E   (M=65: head dim + denominator row)
  out_attn^T = O'^T * (1/denom)      (DVE reads PSUM directly; GPSIMD
                                      partition_broadcast for 1/denom)
  y^T = wproj^T.T @ out_attn^T + bproj

The 64-key tail chunk packs BOTH heads of a pair into one [128, 512]
score tile (head a on partitions 0:64, head b on 64:128, same queries)
so its exp costs half a full chunk; the tail V rows are written twice
(partitions 0:64 and 64:128) by concurrent col-tiled matmuls so each
head's tail AV can run in its own PE row group.

All matmul inputs bf16, PSUM fp32, softmax pipeline fp32.
"""

import numpy as np
import ml_dtypes

B, N, C, H, D = 8, 1024, 1024, 16, 64
P = 128
KC = C // P      # 8 contraction chunks
OC = C // P      # 8 output-channel chunks
NB = N // 512    # 2 query chunks of 512
NK = 576         # padded gathered-key count (max seen 548; +64 margin)
NCORES = 8

_CACHE = {}


DEFAULT_OPTS = {"pj": 2, "sc": 2, "av": 2, "e": 6, "phase": "full",
                "feed": 1, "ilv": 1, "fslots": (0, 1, 2)}


def _build_nc(repeat=1, nk=NK, opts=None):
    import concourse.bass as bass
    import concourse.tile as tile
    from concourse import bacc, mybir
    from contextlib import nullcontext, ExitStack
    opts = {**DEFAULT_OPTS, **(opts or {})}

    bf16 = mybir.dt.bfloat16
    f32 = mybir.dt.float32
    jf = nk // P                  # full key chunks
    tw = nk % P                   # tail width (0 or 64)
    assert tw in (0, 64)
    jt = jf + (1 if tw else 0)    # total chunks

    nc = bacc.Bacc("TRN2", target_bir_lowering=False, debug=False)

    xt_d = nc.dram_tensor("xt", [C, N], bf16, kind="ExternalInput").ap()
    xkv_d = nc.dram_tensor("xkv", [C, nk], bf16, kind="ExternalInput").ap()
    wqt_d = nc.dram_tensor("wqt", [C, C], bf16, kind="ExternalInput").ap()
    wkt_d = nc.dram_tensor("wkt", [C, C], bf16, kind="ExternalInput").ap()
    wvt_d = nc.dram_tensor("wvt", [C, C], bf16, kind="ExternalInput").ap()
    wpt_d = nc.dram_tensor("wpt", [C, C], bf16, kind="ExternalInput").ap()
    mb_d = nc.dram_tensor("mb", [P, jt], f32, kind="ExternalInput").ap()
    bb_d = nc.dram_tensor("bb", [C], f32, kind="ExternalInput").ap()
    yt_d = nc.dram_tensor("yt", [C, N], f32, kind="ExternalOutput").ap()

    with tile.TileContext(nc) as tc:
        with ExitStack() as ctx:
            pools = _make_pools(tc, ctx, opts)
            tiles = _load_inputs(nc, tc, mybir, pools, nk, xt_d, xkv_d, wqt_d,
                                 wkt_d, wvt_d, wpt_d, mb_d, bb_d)
            if repeat > 1:
                from concourse.engine_type import EngineType
                hints = (EngineType.PE, EngineType.Activation, EngineType.DVE,
                         EngineType.Pool, EngineType.SP)
                loop = tc.For_i(0, repeat, 1, hint_engines=hints)
            else:
                loop = nullcontext()
            phase = opts["phase"]
            args = (nc, tc, mybir, pools, nk, tiles, yt_d, opts)
            if phase == "full":
                with loop:
                    _compute(*args)
            else:
                if phase == "qkv":
                    with loop:
                        _compute_qkv(*args)
                    _compute_attn(*args)
                    _compute_proj(*args)
                elif phase == "attn":
                    _compute_qkv(*args)
                    with loop:
                        _compute_attn(*args)
                    _compute_proj(*args)
                elif phase == "proj":
                    _compute_qkv(*args)
                    _compute_attn(*args)
                    with loop:
                        _compute_proj(*args)
                else:
                    raise ValueError(phase)
    nc.compile()
    return nc


def _make_pools(tc, ctx, opts=None):
    opts = {**DEFAULT_OPTS, **(opts or {})}
    return {
        "const": ctx.enter_context(tc.tile_pool(name="const", bufs=1)),
        "e": ctx.enter_context(tc.tile_pool(name="e", bufs=opts["e"])),
        "recip": ctx.enter_context(tc.tile_pool(name="recip", bufs=4)),
        "bcast": ctx.enter_context(tc.tile_pool(name="bcast", bufs=4)),
        "yt": ctx.enter_context(tc.tile_pool(name="yt", bufs=3)),
        "pj_ps": ctx.enter_context(
            tc.tile_pool(name="pj_ps", bufs=opts["pj"], space="PSUM")),
        "sc_ps": ctx.enter_context(
            tc.tile_pool(name="sc_ps", bufs=opts["sc"], space="PSUM")),
        "av_ps": ctx.enter_context(
            tc.tile_pool(name="av_ps", bufs=opts["av"], space="PSUM")),
    }


def _load_inputs(nc, tc, mybir, pools, nk, xt_d, xkv_d, wqt_d, wkt_d, wvt_d,
                 wpt_d, mb_d, bb_d):
    bf16 = mybir.dt.bfloat16
    f32 = mybir.dt.float32
    const = pools["const"]
    jf = nk // P
    tw = nk % P
    jt = jf + (1 if tw else 0)

    xt = const.tile([P, KC, N], bf16)       # x^T   [p, kc, n]
    xkv = const.tile([P, KC, nk], bf16)     # gathered keys x^T [p, kc, j']
    wqt = const.tile([P, KC, C], bf16)      # wq^T  [p, kc, o]  (pre-scaled)
    wkt = const.tile([P, KC, C], bf16)
    wvt = const.tile([P, KC, C], bf16)
    wpt = const.tile([P, KC, C], bf16)
    mb = const.tile([P, jt], f32)           # bias per key slot (0 / -1e9 pad)
    bb = const.tile([P, OC], f32)           # proj bias per out channel o
    qt = const.tile([P, OC, N], bf16)       # q^T [p(o), oc, n]
    kt = const.tile([P, OC, nk], bf16)      # k^T [p(o), oc, j']
    vh = const.tile([P, jt, 65 * H], bf16)  # [p(j'), jc, 65h+dd]; col 64=1
    oa = const.tile([P, KC, N], bf16)       # out_attn^T [p(c), cc, n]

    nc.gpsimd.dma_start(out=xt, in_=xt_d.rearrange("(k p) n -> p k n", p=P))
    nc.gpsimd.dma_start(out=xkv, in_=xkv_d.rearrange("(k p) n -> p k n", p=P))
    nc.gpsimd.dma_start(out=wvt, in_=wvt_d.rearrange("(k p) o -> p k o", p=P))
    nc.gpsimd.dma_start(out=wkt, in_=wkt_d.rearrange("(k p) o -> p k o", p=P))
    nc.gpsimd.dma_start(out=wqt, in_=wqt_d.rearrange("(k p) o -> p k o", p=P))
    nc.gpsimd.dma_start(out=wpt, in_=wpt_d.rearrange("(k p) o -> p k o", p=P))
    nc.gpsimd.dma_start(out=mb, in_=mb_d)
    nc.gpsimd.dma_start(out=bb, in_=bb_d.rearrange("(k p) -> p k", p=P))

    # ones columns of vh (denominator trick), one strided memset per jc
    vh_r = vh.rearrange("p j (h e) -> p j h e", e=65)
    for jc in range(jt):
        nc.vector.memset(vh_r[:, jc, :, 64], 1.0)

    return {"xt": xt, "xkv": xkv, "wqt": wqt, "wkt": wkt, "wvt": wvt,
            "wpt": wpt, "mb": mb, "bb": bb, "qt": qt, "kt": kt, "vh": vh,
            "oa": oa}


def _emit_v_proj(nc, mybir, pools, nk, t):
    """V projection into the interleaved vh layout.  The 64-key tail chunk
    is written twice (partitions 0:64 via col groups 0:1, partitions 64:128
    via col groups 2:3 -- concurrent col-tiled matmuls) so both heads of a
    pair can later run their tail AV in separate PE row groups."""
    f32 = mybir.dt.float32
    xkv, wvt, vh = t["xkv"], t["wvt"], t["vh"]
    pj_ps = pools["pj_ps"]
    jf = nk // P
    tw = nk % P
    for nb2 in range(2):
        for mc in range(jf):
            ps = pj_ps.tile([P, 512], f32, name=f"v_{nb2}_{mc}", tag="pj")
            for kc in range(KC):
                nc.tensor.matmul(
                    ps, xkv[:, kc, mc * P:(mc + 1) * P],
                    wvt[:, kc, nb2 * 512:(nb2 + 1) * 512],
                    start=(kc == 0), stop=(kc == KC - 1))
            for hh in range(8):
                h = nb2 * 8 + hh
                nc.vector.tensor_copy(
                    vh[:, mc, 65 * h:65 * h + 64],
                    ps[:, hh * 64:(hh + 1) * 64])
        if tw:
            ps = pj_ps.tile([P, 512], f32, name=f"vt_{nb2}", tag="pj")
            for kc in range(KC):
                nc.tensor.matmul(
                    ps[0:tw, :], xkv[:, kc, jf * P:jf * P + tw],
                    wvt[:, kc, nb2 * 512:(nb2 + 1) * 512],
                    start=(kc == 0), stop=(kc == KC - 1))
                nc.tensor.matmul(
                    ps[64:64 + tw, :], xkv[:, kc, jf * P:jf * P + tw],
                    wvt[:, kc, nb2 * 512:(nb2 + 1) * 512],
                    start=(kc == 0), stop=(kc == KC - 1))
            for hh in range(8):
                h = nb2 * 8 + hh
                # rows 0:64 and 64:128 both hold the tail keys' V (dup)
                nc.vector.tensor_copy(
                    vh[:, jf, 65 * h:65 * h + 64],
                    ps[:, hh * 64:(hh + 1) * 64])


def _qk_units(nc, mybir, pools, nk, t, g):
    """Yield fine-grained PE work units (4-matmul halves) for head pair g's
    Q and K projections, to be interleaved into attention exp-wait slots."""
    f32 = mybir.dt.float32
    xt, xkv, wqt, wkt = t["xt"], t["xkv"], t["wqt"], t["wkt"]
    qt, kt = t["qt"], t["kt"]
    pj_ps = pools["pj_ps"]
    jf = nk // P
    tw = nk % P

    def q_half(nb2, half, ps_box):
        def emit():
            if half == 0:
                ps_box[0] = pj_ps.tile([P, 512], f32, name=f"q_{g}_{nb2}",
                                       tag="pj")
            ps = ps_box[0]
            for kc in range(4 * half, 4 * half + 4):
                nc.tensor.matmul(
                    ps, wqt[:, kc, g * P:(g + 1) * P],
                    xt[:, kc, nb2 * 512:(nb2 + 1) * 512],
                    start=(kc == 0), stop=(kc == KC - 1))
            if half == 1:
                nc.vector.tensor_copy(qt[:, g, nb2 * 512:(nb2 + 1) * 512], ps)
        return emit

    def k_half(half, ps_box):
        def emit():
            if half == 0:
                ps_box[0] = pj_ps.tile([P, 512], f32, name=f"k_{g}",
                                       tag="pj")
            ps = ps_box[0]
            for kc in range(4 * half, 4 * half + 4):
                nc.tensor.matmul(
                    ps, wkt[:, kc, g * P:(g + 1) * P],
                    xkv[:, kc, 0:512],
                    start=(kc == 0), stop=(kc == KC - 1))
            if half == 1:
                nc.vector.tensor_copy(kt[:, g, 0:512], ps)
        return emit

    def k_tail():
        ps = pj_ps.tile([P, 64], f32, name=f"kt_{g}", tag="pj")
        for kc in range(KC):
            nc.tensor.matmul(
                ps, wkt[:, kc, g * P:(g + 1) * P],
                xkv[:, kc, jf * P:jf * P + tw],
                start=(kc == 0), stop=(kc == KC - 1))
        nc.vector.tensor_copy(kt[:, g, jf * P:jf * P + tw], ps)

    for nb2 in range(NB):
        box = [None]
        yield q_half(nb2, 0, box)
        yield q_half(nb2, 1, box)
    box = [None]
    yield k_half(0, box)
    yield k_half(1, box)
    if tw:
        yield k_tail


def _attn_group(nc, mybir, pools, nk, t, g, ic, opts, feeder=None):
    """Attention for head pair (2g, 2g+1), query chunk ic (512 queries).
    Score pairs run concurrently in PE row groups 0:64 / 64:128; the merged
    [128, 1024] exp covers both heads; AV accumulates [65, 512] per head
    (64 dims + denominator row); the tail chunk packs both heads into one
    [128, 512] score tile.  Normalization reads the AV PSUM directly."""
    bf16 = mybir.dt.bfloat16
    f32 = mybir.dt.float32
    Exp = mybir.ActivationFunctionType.Exp
    mb, qt, kt, vh, oa = t["mb"], t["qt"], t["kt"], t["vh"], t["oa"]
    e_pool, r_pool, bc_pool = pools["e"], pools["recip"], pools["bcast"]
    sc_ps, av_ps = pools["sc_ps"], pools["av_ps"]
    jf = nk // P
    tw = nk % P
    jt = jf + (1 if tw else 0)
    ha, hb = 2 * g, 2 * g + 1
    i0 = ic * 512

    avs = {}
    for h in (ha, hb):
        avs[h] = av_ps.tile([65, 512], f32, name=f"av_{h}_{ic}", tag="av")
    edict = {}

    def scores_chunk(jc):
        if jc < jf:
            s2 = sc_ps.tile([P, 1024], f32, name=f"s2_{g}_{ic}_{jc}",
                            tag="s2")
            for (h, p0), c0 in (((ha, 0), 0), ((hb, 64), 512)):
                nc.tensor.matmul(
                    s2[:, c0:c0 + 512],
                    kt[p0:p0 + 64, g, jc * P:(jc + 1) * P],
                    qt[p0:p0 + 64, g, i0:i0 + 512],
                    start=True, stop=True)
            e2 = e_pool.tile([P, 1024], bf16, name=f"e2_{g}_{ic}_{jc}",
                             tag="e")
            nc.scalar.activation(e2, s2, Exp, bias=mb[:, jc:jc + 1])
            edict[jc] = e2
        else:
            # tail: head a keys on partitions 0:64, head b on 64:128,
            # both over the same 512 queries -> one half-width exp
            st = sc_ps.tile([P, 512], f32, name=f"st_{g}_{ic}", tag="s2")
            for h, p0 in ((ha, 0), (hb, 64)):
                nc.tensor.matmul(
                    st[p0:p0 + tw, :],
                    kt[p0:p0 + 64, g, jf * P:jf * P + tw],
                    qt[p0:p0 + 64, g, i0:i0 + 512],
                    start=True, stop=True)
            et = e_pool.tile([P, 512], bf16, name=f"et_{g}_{ic}", tag="e")
            nc.scalar.activation(et, st, Exp, bias=mb[:, jf:jf + 1])
            edict[jc] = et

    def av_chunk(jc):
        e = edict.pop(jc)
        first = jc == 0
        last = jc == jt - 1
        if jc < jf:
            for h, c0 in ((ha, 0), (hb, 512)):
                nc.tensor.matmul(
                    avs[h], vh[:, jc, 65 * h:65 * h + 65],
                    e[:, c0:c0 + 512], start=first, stop=last)
        else:
            # tail AVs run concurrent: head a in rows 0:64, head b (using
            # the duplicated tail V rows) in rows 64:128
            for h, p0 in ((ha, 0), (hb, 64)):
                nc.tensor.matmul(
                    avs[h], vh[p0:p0 + tw, jf, 65 * h:65 * h + 65],
                    e[p0:p0 + tw, :], start=first, stop=last)

    def feed():
        if feeder is not None:
            feeder()

    fslots = opts["fslots"]
    scores_chunk(0)
    for jc in range(jt):
        if jc + 1 < jt:
            scores_chunk(jc + 1)
        if jc in fslots:
            feed()
        av_chunk(jc)

    for h, p0 in ((ha, 0), (hb, 64)):
        recip = r_pool.tile([1, 512], f32, name=f"r_{h}_{ic}", tag="r")
        nc.vector.reciprocal_approx_fast(recip, avs[h][64:65, :])
        bc = bc_pool.tile([64, 512], f32, name=f"bc_{h}_{ic}", tag="bc")
        nc.gpsimd.partition_broadcast(bc, recip)
        nc.vector.tensor_mul(
            oa[p0:p0 + 64, g, i0:i0 + 512], avs[h][0:64, :], bc)


def _compute_qkv(nc, tc, mybir, pools, nk, t, yt_d, opts=None):
    opts = {**DEFAULT_OPTS, **(opts or {})}
    _emit_v_proj(nc, mybir, pools, nk, t)
    for g in range(OC):
        for unit in _qk_units(nc, mybir, pools, nk, t, g):
            unit()


def _compute_attn(nc, tc, mybir, pools, nk, t, yt_d, opts=None):
    opts = {**DEFAULT_OPTS, **(opts or {})}
    for g in range(OC):
        for ic in range(NB):
            _attn_group(nc, mybir, pools, nk, t, g, ic, opts)


def _compute_proj(nc, tc, mybir, pools, nk, t, yt_d, opts=None):
    opts = {**DEFAULT_OPTS, **(opts or {})}
    f32 = mybir.dt.float32
    wpt, oa, bb = t["wpt"], t["oa"], t["bb"]
    pj_ps, y_pool = pools["pj_ps"], pools["yt"]
    for oc in range(OC):
        for nb2 in range(NB):
            ps = pj_ps.tile([P, 512], f32, name=f"y_{oc}_{nb2}", tag="pj")
            for kc in range(KC):
                nc.tensor.matmul(
                    ps, wpt[:, kc, oc * P:(oc + 1) * P],
                    oa[:, kc, nb2 * 512:(nb2 + 1) * 512],
                    start=(kc == 0), stop=(kc == KC - 1))
            yt = y_pool.tile([P, 512], f32, name=f"yt_{oc}_{nb2}", tag="yt")
            nc.vector.tensor_scalar_add(yt, ps, bb[:, oc:oc + 1])
            nc.gpsimd.dma_start(
                out=yt_d[oc * P:(oc + 1) * P, nb2 * 512:(nb2 + 1) * 512],
                in_=yt)


def _compute(nc, tc, mybir, pools, nk, t, yt_d, opts=None):
    """Production emission order: V projection + first two pairs' Q/K, then
    attention groups interleaved two-pairs-at-a-time (so AV PSUM banks have
    two group-slots to drain) with later pairs' Q/K projections fed into
    exp-wait slots, then the out-projection."""
    opts = {**DEFAULT_OPTS, **(opts or {})}

    _emit_v_proj(nc, mybir, pools, nk, t)
    for g in (0, 1):
        for unit in _qk_units(nc, mybir, pools, nk, t, g):
            unit()

    if opts["ilv"]:
        order = []
        for gg in range(0, OC, 2):
            for ic in range(NB):
                for g in (gg, gg + 1):
                    order.append((g, ic))
    else:
        order = [(g, ic) for g in range(OC) for ic in range(NB)]

    # feeder supplies later pairs' Q/K units while earlier attention runs;
    # ensure() guarantees a pair's Q/K is fully emitted before its scores
    feed_lists = {g: list(_qk_units(nc, mybir, pools, nk, t, g))
                  for g in range(2, OC)}
    units = []
    done_upto = {}
    for g in range(2, OC):
        units.extend(feed_lists[g])
        done_upto[g] = len(units)
    pos = [0]

    def feed_one():
        if pos[0] < len(units):
            units[pos[0]]()
            pos[0] += 1

    def ensure(g):
        target = done_upto.get(g, 0)
        while pos[0] < target:
            units[pos[0]]()
            pos[0] += 1

    for g, ic in order:
        ensure(g)
        _attn_group(nc, mybir, pools, nk, t, g, ic, opts,
                    feeder=feed_one if opts["feed"] else None)
    ensure(OC - 1)

    _compute_proj(nc, tc, mybir, pools, nk, t, yt_d, opts)


def _prep_inputs(x, mask, wq, wk, wv, wproj, bproj, nk=NK):
    """Host-side preprocessing: key gathering, transposes, scaling, casts."""
    bf = ml_dtypes.bfloat16
    scale = D ** (-0.5)
    jf = nk // P
    tw = nk % P
    jt = jf + (1 if tw else 0)
    wqt = np.ascontiguousarray((np.asarray(wq) * scale).T).astype(bf)
    wkt = np.ascontiguousarray(np.asarray(wk).T).astype(bf)
    wvt = np.ascontiguousarray(np.asarray(wv).T).astype(bf)
    wpt = np.ascontiguousarray(np.asarray(wproj).T).astype(bf)
    bb = np.ascontiguousarray(np.asarray(bproj, dtype=np.float32))
    x = np.asarray(x)
    full_mask = np.concatenate(
        [np.ones((B, 1), dtype=bool), np.asarray(mask)], axis=1)
    in_maps = []
    for b in range(B):
        xt = np.ascontiguousarray(x[b].T).astype(bf)
        idx = np.flatnonzero(full_mask[b])
        nk_b = idx.size
        assert nk_b <= nk, f"batch {b}: {nk_b} unmasked keys > padded {nk}"
        xg = np.zeros((nk, C), np.float32)
        xg[:nk_b] = x[b][idx]
        xkv = np.ascontiguousarray(xg.T).astype(bf)
        bias = np.full(nk, -1e9, np.float32)
        bias[:nk_b] = 0.0
        mb = np.empty((P, jt), np.float32)
        for jc in range(jf):
            mb[:, jc] = bias[jc * P:(jc + 1) * P]
        if tw:
            mb[0:tw, jf] = bias[jf * P:jf * P + tw]
            mb[64:64 + tw, jf] = bias[jf * P:jf * P + tw]
        in_maps.append({
            "xt": xt, "xkv": xkv, "wqt": wqt, "wkt": wkt, "wvt": wvt,
            "wpt": wpt, "mb": mb, "bb": bb,
        })
    return in_maps


def get_nc(repeat=1, nk=NK, opts=None):
    key = ("nc", repeat, nk, tuple(sorted((opts or {}).items())))
    if key not in _CACHE:
        _CACHE[key] = _build_nc(repeat, nk, opts)
    return _CACHE[key]


def kernel(x, mask, wq, wk, wv, wproj, bproj):
    from concourse.bass_utils import run_bass_kernel_spmd
    full_mask = np.concatenate(
        [np.ones((B, 1), dtype=bool), np.asarray(mask)], axis=1)
    max_nk = int(full_mask.sum(axis=1).max())
    nk = NK if max_nk <= NK else ((max_nk + 63) // 64) * 64
    nc = get_nc(nk=nk)
    in_maps = _prep_inputs(x, mask, wq, wk, wv, wproj, bproj, nk=nk)
    res = run_bass_kernel_spmd(nc, in_maps, core_ids=list(range(NCORES)))
    out = np.empty((B, N, C), np.float32)
    for b in range(B):
        out[b] = res.results[b]["yt"].T
    return out


if __name__ == "__main__":
    rng = np.random.default_rng(0)
    ins = {
        "x": rng.standard_normal((B, N, C), dtype=np.float32),
        "mask": rng.integers(0, 2, (B, N - 1)).astype(bool),
        "wq": rng.standard_normal((C, C), dtype=np.float32) * 0.02,
        "wk": rng.standard_normal((C, C), dtype=np.float32) * 0.02,
        "wv": rng.standard_normal((C, C), dtype=np.float32) * 0.02,
        "wproj": rng.standard_normal((C, C), dtype=np.float32) * 0.02,
        "bproj": rng.standard_normal((C,), dtype=np.float32) * 0.02,
    }
    o = kernel(**ins)
    print(o.shape, o.dtype)
